# revision 26
# baseline (speedup 1.0000x reference)
"""Distributed Trainium2 kernel for nn_AverageBackProjection (sparse 3-conv chain).

Strategy:
  - Host: reorder voxels with reverse-Cuthill-McKee so graph neighbors are close
    in index space; fold the residual projection (x - upsample(mean)) into W0;
    shard the voxel dim across 8 cores with replicated halo compute (no
    collectives); precompute int16 ring-slot gather indices for every tile.
  - Device (per core, SPMD): three conv phases. Each phase streams a channel-
    major window of the source rows through an SBUF ring buffer, fans out
    neighbor rows with gpsimd ap_gather, and accumulates 27 (or 7 packed)
    matmuls per 512-voxel tile in PSUM. Final phase adds the downsampled mean.
"""

import os
import sys

sys.path.insert(0, "/opt/trn_rl_repo")
os.environ.setdefault("JAX_COMPILATION_CACHE_DIR", "/tmp/jax_cache")
os.environ.setdefault("JAX_PERSISTENT_CACHE_MIN_COMPILE_TIME_SECS", "10")
os.environ.setdefault("JAX_PERSISTENT_CACHE_MIN_ENTRY_SIZE_BYTES", "0")

import numpy as np

N = 300000
NC = 8
NS = N // NC              # 37500 rows per core
C = 128
CO = 32
K = 27
T = 1024                  # tile (voxels per gather batch)
MM = 512                  # matmul moving-dim max (2 matmuls per gather tile)
H = 3584                  # halo reach (>= max neighbor distance after RCM)
R = 10240                 # ring data slots (10*T)
ZSLOT = R                 # zero-row slot
NE = R + 1
PRE = 2 * H + T           # 8192 ring preload cols
L2 = 37 * T               # 37888 conv2 rows (own shard, padded)
L1 = L2 + 2 * H           # 45056 conv1 rows
L0 = L1 + 2 * H           # 52224 conv0 rows
LW = L0 + 2 * H           # 59392 feats window cols
NT2 = L2 // T
NT1 = L1 // T
NT0 = L0 // T
NJ = (K + 3) // 4         # 7 packed index groups for conv2

LAST_EXEC_NS = None
LAST_RESULTS = None


def _perm_rcm(nbr_idx):
    from scipy.sparse import coo_matrix
    from scipy.sparse.csgraph import reverse_cuthill_mckee

    k_idx, n_idx = np.nonzero(nbr_idx < N)
    src = nbr_idx[k_idx, n_idx].astype(np.int64)
    dst = n_idx.astype(np.int64)
    a = coo_matrix((np.ones(len(src), np.int8), (dst, src)), shape=(N, N)).tocsr()
    perm = np.asarray(reverse_cuthill_mckee(a, symmetric_mode=True), dtype=np.int64)
    return perm


def _preprocess(feats, W0, W1, W2, nbr_idx):
    feats = np.asarray(feats, np.float32)
    W0 = np.asarray(W0, np.float32)
    W1 = np.asarray(W1, np.float32)
    W2 = np.asarray(W2, np.float32)
    nbr_idx = np.asarray(nbr_idx)

    perm = _perm_rcm(nbr_idx)
    inv = np.empty(N, np.int64)
    inv[perm] = np.arange(N)

    # remapped neighbor table: nbr_new[k, i] = new index of k-neighbor of new-i
    v = nbr_idx[:, perm]
    valid = v < N
    nbr_new = np.where(valid, inv[np.clip(v, 0, N - 1)], np.int64(-1))

    d = np.abs(nbr_new - np.arange(N)[None, :])
    maxdist = int(d[nbr_new >= 0].max()) if (nbr_new >= 0).any() else 0
    if maxdist > H:
        raise RuntimeError(
            f"neighbor distance {maxdist} exceeds halo {H}; ring kernel invalid"
        )

    # fold residual projection A = I - (1/4) * kron(ones(4,4), eye(32)) into W0
    A = np.eye(C, dtype=np.float32) - np.kron(
        np.ones((C // CO, C // CO), np.float32), np.eye(CO, dtype=np.float32)
    ) / (C // CO)
    W0p = np.einsum("ce,kem->kcm", A, W0).astype(np.float32)
    W0p = np.ascontiguousarray(W0p)

    # packed conv2 weights: group j holds k = 4j..4j+3 stacked on partitions
    W2s = np.zeros((NJ, C, CO), np.float32)
    for j in range(NJ):
        for i in range(4):
            k = 4 * j + i
            if k < K:
                W2s[j, 32 * i:32 * i + 32, :] = W2[k]

    # feats window, channel-major, padded on both global ends
    padl = 3 * H
    padr = LW                      # generous right pad
    fw = np.zeros((C, padl + N + padr), np.float32)
    fw[:, padl:padl + N] = feats[perm].T

    def idx_for(conv_off, L, nt):
        """Ring-slot indices [K, L] for rows g = r*NS - conv_off + j."""
        out = np.empty((NC, nt, 16, K, T // 16), np.int16)
        j = np.arange(L, dtype=np.int64)
        for r in range(NC):
            g = r * NS - conv_off + j
            inb = (g >= 0) & (g < N)
            gc = np.clip(g, 0, N - 1)
            slots = np.empty((K, L), np.int64)
            for k in range(K):
                ns = nbr_new[k, gc]
                val = inb & (ns >= 0)
                # ring coord = (ns - (r*NS - conv_off - H)) ; slot = coord % R
                coord = ns - (r * NS - conv_off - H)
                slots[k] = np.where(val, coord % R, ZSLOT)
            # layout [nt, 16(p), K, T//16(s)]: idx j = s*16 + p
            out[r] = slots.reshape(K, nt, T // 16, 16).transpose(1, 3, 0, 2)
        return out.astype(np.int16)

    idx0 = idx_for(2 * H, L0, NT0)          # [NC, NT0, 16, K, 32]
    idx1 = idx_for(H, L1, NT1)
    idx2u = idx_for(0, L2, NT2)             # unpacked [NC, NT2, 16, K, 32]
    # pack conv2, pre-replicated across partitions:
    # [NC, NT2, 128, NJ, 32]; partition 32k+16rep+p carries k-group member k
    idx2 = np.full((NC, NT2, C, NJ, T // 16), ZSLOT, np.int16)
    for j in range(NJ):
        for i in range(4):
            k = 4 * j + i
            if k < K:
                for rep in range(2):
                    idx2[:, :, 32 * i + 16 * rep:32 * i + 16 * rep + 16, j] = \
                        idx2u[:, :, :, k]

    in_maps = []
    for r in range(NC):
        base = padl + r * NS - 3 * H
        in_maps.append({
            "featsw": np.ascontiguousarray(fw[:, base:base + LW]),
            "idx0": np.ascontiguousarray(idx0[r]).reshape(NT0, 16, -1),
            "idx1": np.ascontiguousarray(idx1[r]).reshape(NT1, 16, -1),
            "idx2": np.ascontiguousarray(idx2[r]).reshape(NT2, C, -1),
            "w0": W0p,
            "w1": np.ascontiguousarray(W1),
            "w2": np.ascontiguousarray(W2s),
            "wd": np.ascontiguousarray(
                np.kron(np.ones((C // CO, 1), np.float32),
                        np.eye(CO, dtype=np.float32)) / (C // CO)
            ),
        })
    return in_maps, perm


def _build_graph():
    import concourse.bacc as bacc
    import concourse.mybir as mybir
    import concourse.tile as tile

    F32 = mybir.dt.float32
    I16 = mybir.dt.int16

    nc = bacc.Bacc(None, target_bir_lowering=False, debug=False)
    featsw = nc.declare_dram_parameter("featsw", [C, LW], F32, isOutput=False)
    idx0 = nc.declare_dram_parameter("idx0", [NT0, 16, K * (T // 16)], I16, isOutput=False)
    idx1 = nc.declare_dram_parameter("idx1", [NT1, 16, K * (T // 16)], I16, isOutput=False)
    idx2 = nc.declare_dram_parameter("idx2", [NT2, C, NJ * (T // 16)], I16, isOutput=False)
    w0 = nc.declare_dram_parameter("w0", [K, C, C], F32, isOutput=False)
    w1 = nc.declare_dram_parameter("w1", [K, C, CO], F32, isOutput=False)
    w2 = nc.declare_dram_parameter("w2", [NJ, C, CO], F32, isOutput=False)
    wd = nc.declare_dram_parameter("wd", [C, CO], F32, isOutput=False)
    out_ext = nc.declare_dram_parameter("out", [CO, L2], F32, isOutput=True)

    out0_win = nc.dram_tensor("out0_win", [C, L0], F32)
    out1_win = nc.dram_tensor("out1_win", [CO, L1], F32)
    dT_dram = nc.dram_tensor("dT_dram", [CO, L2], F32)

    with tile.TileContext(nc) as tc:
        with (
            tc.tile_pool(name="pers", bufs=1) as pers,
            tc.tile_pool(name="idxp", bufs=4) as idxp,
            tc.tile_pool(name="gp", bufs=4) as gp,
            tc.tile_pool(name="op", bufs=3) as op,
            tc.tile_pool(name="dtp", bufs=3) as dtp,
            tc.tile_pool(name="ps0", bufs=2, space="PSUM") as ps0p,
            tc.tile_pool(name="psd", bufs=2, space="PSUM") as psdp,
        ):
            # ---------------- phase 0: conv0 (feats -> out0_win) -------------
            ws0 = pers.tile([C, K, C], F32, tag="ws0")
            nc.sync.dma_start(out=ws0[:, :, :], in_=w0.ap().rearrange("k c m -> c k m"))
            wsd = pers.tile([C, CO], F32, tag="wsd")
            nc.sync.dma_start(out=wsd[:, :], in_=wd[:, :])
            ring = pers.tile([C, NE], F32, tag="ring")
            nc.vector.memset(ring[:, R:NE], 0.0)
            nc.sync.dma_start(out=ring[:, 0:PRE], in_=featsw[:, 0:PRE])
            for t in range(NT0):
                if t > 0:
                    c0 = PRE + (t - 1) * T
                    s0 = c0 % R
                    nc.sync.dma_start(
                        out=ring[:, s0:s0 + T], in_=featsw[:, c0:c0 + T]
                    )
                idxs = idxp.tile([C, K, T // 16], I16, tag="idx")
                src = idx0[t].unsqueeze(0).broadcast_to([8, 16, K * (T // 16)])
                nc.sync.dma_start(
                    out=idxs[:, :, :].rearrange("p k s -> p (k s)"), in_=src
                )
                psum = ps0p.tile([C, T], F32, tag="acc0")
                for k in range(K):
                    g = gp.tile([C, T], F32, tag="g")
                    nc.gpsimd.ap_gather(
                        out_ap=g[:, :], in_ap=ring[:, :], idxs_ap=idxs[:, k, :],
                        channels=C, num_elems=NE, d=1, num_idxs=T,
                    )
                    for h in range(T // MM):
                        nc.tensor.matmul(
                            psum[:, h * MM:(h + 1) * MM], ws0[:, k, :],
                            g[:, h * MM:(h + 1) * MM],
                            start=(k == 0), stop=(k == K - 1),
                        )
                # downsampled mean for own rows (tiles 7..7+NT2) via PE:
                # dT = wd.T @ ring_slice  (wd = 0.25 * stacked eye(32))
                if 2 * H // T <= t < 2 * H // T + NT2:
                    c = t - 2 * H // T
                    for h in range(2):
                        sl = (c * T + 3 * H + h * 512) % R
                        psd = psdp.tile([CO, MM], F32, tag="accd")
                        nc.tensor.matmul(
                            psd[:, :], wsd[:, :], ring[:, sl:sl + 512],
                            start=True, stop=True,
                        )
                        t2 = dtp.tile([CO, MM], F32, tag="dt2")
                        nc.vector.tensor_copy(t2[:, :], psd[:, :])
                        nc.sync.dma_start(
                            out=dT_dram[:, c * T + h * 512:c * T + (h + 1) * 512],
                            in_=t2[:, :],
                        )
                oc = op.tile([C, T], F32, tag="oc0")
                nc.vector.tensor_copy(oc[:, :], psum[:, :])
                nc.sync.dma_start(out=out0_win[:, t * T:(t + 1) * T], in_=oc[:, :])

            # ---------------- phase 1: conv1 (out0_win -> out1_win) ----------
            ws1 = pers.tile([C, K, CO], F32, tag="ws1")
            nc.sync.dma_start(out=ws1[:, :, :], in_=w1.ap().rearrange("k c m -> c k m"))
            ring1 = pers.tile([C, NE], F32, tag="ring")
            nc.vector.memset(ring1[:, R:NE], 0.0)
            nc.sync.dma_start(out=ring1[:, 0:PRE], in_=out0_win[:, 0:PRE])
            for t in range(NT1):
                if t > 0:
                    c0 = PRE + (t - 1) * T
                    s0 = c0 % R
                    nc.sync.dma_start(
                        out=ring1[:, s0:s0 + T], in_=out0_win[:, c0:c0 + T]
                    )
                idxs = idxp.tile([C, K, T // 16], I16, tag="idx")
                src = idx1[t].unsqueeze(0).broadcast_to([8, 16, K * (T // 16)])
                nc.sync.dma_start(
                    out=idxs[:, :, :].rearrange("p k s -> p (k s)"), in_=src
                )
                psum = ps0p.tile([CO, T], F32, tag="acc0")
                for k in range(K):
                    g = gp.tile([C, T], F32, tag="g")
                    nc.gpsimd.ap_gather(
                        out_ap=g[:, :], in_ap=ring1[:, :], idxs_ap=idxs[:, k, :],
                        channels=C, num_elems=NE, d=1, num_idxs=T,
                    )
                    for h in range(T // MM):
                        nc.tensor.matmul(
                            psum[:, h * MM:(h + 1) * MM], ws1[:, k, :],
                            g[:, h * MM:(h + 1) * MM],
                            start=(k == 0), stop=(k == K - 1),
                        )
                oc = op.tile([CO, T], F32, tag="oc1")
                nc.vector.tensor_copy(oc[:, :], psum[:, :])
                nc.sync.dma_start(out=out1_win[:, t * T:(t + 1) * T], in_=oc[:, :])

            # ---------------- phase 2: conv2 (out1_win + dT -> out) ----------
            ws2 = pers.tile([C, NJ, CO], F32, tag="ws2")
            nc.sync.dma_start(out=ws2[:, :, :], in_=w2.ap().rearrange("k c m -> c k m"))
            ring2 = pers.tile([C, NE], F32, tag="ring")
            nc.vector.memset(ring2[:, R:NE], 0.0)
            for i in range(4):
                nc.sync.dma_start(
                    out=ring2[32 * i:32 * i + 32, 0:PRE], in_=out1_win[:, 0:PRE]
                )
            for t in range(NT2):
                if t > 0:
                    c0 = PRE + (t - 1) * T
                    s0 = c0 % R
                    for i in range(4):
                        nc.sync.dma_start(
                            out=ring2[32 * i:32 * i + 32, s0:s0 + T],
                            in_=out1_win[:, c0:c0 + T],
                        )
                idxs = idxp.tile([C, NJ, T // 16], I16, tag="idx")
                nc.sync.dma_start(
                    out=idxs[:, :, :].rearrange("p j s -> p (j s)"), in_=idx2[t]
                )
                psum = ps0p.tile([CO, T], F32, tag="acc0")
                for j in range(NJ):
                    g = gp.tile([C, T], F32, tag="g")
                    nc.gpsimd.ap_gather(
                        out_ap=g[:, :], in_ap=ring2[:, :], idxs_ap=idxs[:, j, :],
                        channels=C, num_elems=NE, d=1, num_idxs=T,
                    )
                    for h in range(T // MM):
                        nc.tensor.matmul(
                            psum[:, h * MM:(h + 1) * MM], ws2[:, j, :],
                            g[:, h * MM:(h + 1) * MM],
                            start=(j == 0), stop=(j == NJ - 1),
                        )
                dtt = dtp.tile([CO, T], F32, tag="dt1")
                nc.sync.dma_start(out=dtt[:, :], in_=dT_dram[:, t * T:(t + 1) * T])
                oc = op.tile([CO, T], F32, tag="oc1")
                nc.vector.tensor_add(oc[:, :], psum[:, :], dtt[:, :])
                nc.sync.dma_start(out=out_ext[:, t * T:(t + 1) * T], in_=oc[:, :])

    nc.compile()
    return nc


def kernel(feats, W0, W1, W2, nbr_idx):
    global LAST_EXEC_NS, LAST_RESULTS
    from concourse.bass_utils import run_bass_kernel_spmd

    in_maps, perm = _preprocess(feats, W0, W1, W2, nbr_idx)
    nc = _build_graph()
    trace = os.environ.get("KERNEL_NO_TRACE", "") == ""
    res = run_bass_kernel_spmd(
        nc, in_maps, core_ids=list(range(NC)), trace=trace
    )
    LAST_EXEC_NS = res.exec_time_ns
    LAST_RESULTS = res

    out_p = np.empty((N, CO), np.float32)
    for r in range(NC):
        out_p[r * NS:(r + 1) * NS] = res.results[r]["out"][:, :NS].T
    out = np.empty((N, CO), np.float32)
    out[perm] = out_p
    return out


# revision 28
# speedup vs baseline: 2.1096x; 2.1096x over previous
"""Distributed Trainium2 kernel for nn_AverageBackProjection (sparse 3-conv chain).

Strategy:
  - Host: reverse-Cuthill-McKee voxel reordering (neighbors land within +-3584
    positions); residual projection folded into W0; voxel dim sharded across 8
    cores with replicated-halo compute (zero collectives); per-tile int16
    ring-slot gather indices precomputed.
  - Device (per core, SPMD, 3 phases): source rows stream through an SBUF
    "token ring" (one 256B bf16 row per token).  Fan-out of the 27 neighbor
    offsets is done by gpsimd dma_gather (SWDGE descriptors, all 16 DMA
    engines) in transpose mode, which yields channel-major [128, 896] bf16
    tiles feeding PSUM-accumulated bf16 matmuls.  Conv outputs are cast to
    bf16, transposed to row-major via the DMA XBAR, and written to DRAM to
    feed the next phase's ring.  The downsampled mean rides on the identity
    (k=13) gather through a tiny selection matmul.
"""

import os
import sys

sys.path.insert(0, "/opt/trn_rl_repo")
os.environ.setdefault("JAX_COMPILATION_CACHE_DIR", "/tmp/jax_cache")
os.environ.setdefault("JAX_PERSISTENT_CACHE_MIN_COMPILE_TIME_SECS", "10")
os.environ.setdefault("JAX_PERSISTENT_CACHE_MIN_ENTRY_SIZE_BYTES", "0")

import numpy as np

N = 300000
NC = 8
NS = N // NC              # 37500 rows per core
C = 128
CO = 32
K = 27
T = 896                   # gather batch (max under the 64-desc SWDGE ring cap)
H = 3584                  # halo reach = 4*T (>= max RCM neighbor distance)
SPAN = 2 * H + T          # 8064 window span per tile
R = 8960                  # ring data tokens = 10*T; zero token at slot R
NE = R + 1
STR = R // 128            # 70 data stripes; zero token in stripe 70
NT2 = 42                  # own-shard tiles  (L2 = 37632 >= 37500)
NT1 = 50                  # conv1 tiles      (L1 = 44800 = L2 + 2H)
NT0 = 58                  # conv0 tiles      (L0 = 51968 = L1 + 2H)
NTW = 66                  # feats window     (LW = 59136 = L0 + 2H)
L2, L1, L0, LW = NT2 * T, NT1 * T, NT0 * T, NTW * T
SKEW = 2 * H // T         # 8: conv0 tile (c+8) holds own-row block c

LAST_EXEC_NS = None
LAST_RESULTS = None


def _bf16(x):
    import ml_dtypes
    return np.asarray(x, dtype=ml_dtypes.bfloat16)


def _perm_rcm(nbr_idx):
    from scipy.sparse import coo_matrix
    from scipy.sparse.csgraph import reverse_cuthill_mckee

    k_idx, n_idx = np.nonzero(nbr_idx < N)
    src = nbr_idx[k_idx, n_idx].astype(np.int64)
    dst = n_idx.astype(np.int64)
    a = coo_matrix((np.ones(len(src), np.int8), (dst, src)), shape=(N, N)).tocsr()
    return np.asarray(reverse_cuthill_mckee(a, symmetric_mode=True), dtype=np.int64)


def _preprocess(feats, W0, W1, W2, nbr_idx):
    feats = np.asarray(feats, np.float32)
    W0 = np.asarray(W0, np.float32)
    W1 = np.asarray(W1, np.float32)
    W2 = np.asarray(W2, np.float32)
    nbr_idx = np.asarray(nbr_idx)

    perm = _perm_rcm(nbr_idx)
    inv = np.empty(N, np.int64)
    inv[perm] = np.arange(N)

    v = nbr_idx[:, perm]
    nbr_new = np.where(v < N, inv[np.clip(v, 0, N - 1)], np.int64(-1))

    d = np.abs(nbr_new - np.arange(N)[None, :])
    maxdist = int(d[nbr_new >= 0].max()) if (nbr_new >= 0).any() else 0
    if maxdist > H:
        raise RuntimeError(f"neighbor distance {maxdist} exceeds halo {H}")

    # fold residual projection A = I - upsample(mean) into W0
    A = np.eye(C, dtype=np.float32) - np.kron(
        np.ones((C // CO, C // CO), np.float32), np.eye(CO, dtype=np.float32)
    ) / (C // CO)
    W0p = np.einsum("ce,kem->kcm", A, W0).astype(np.float32)

    # conv2 weights padded to 128 contraction rows (gathered pad chans = 0)
    W2p = np.zeros((K, C, CO), np.float32)
    W2p[:, :CO, :] = W2

    wd = np.kron(np.ones((C // CO, 1), np.float32),
                 np.eye(CO, dtype=np.float32)) / (C // CO)

    # feats window rows (row-major), bf16, zero-padded outside [0, N)
    feats_p = feats[perm]

    def idx_for(conv_off, nt):
        """Wrapped int16 ring slots [NC, nt, 16, T//16].

        Tile t of this conv covers g = r*NS - conv_off + tT + s; source ns maps
        to ring coord = ns - (r*NS - conv_off - H), slot = coord % R."""
        out = np.empty((NC, nt, 16, K, T // 16), np.int16)
        j = np.arange(nt * T, dtype=np.int64)
        for r in range(NC):
            g = r * NS - conv_off + j
            inb = (g >= 0) & (g < N)
            gc = np.clip(g, 0, N - 1)
            slots = np.empty((K, nt * T), np.int64)
            for k in range(K):
                ns = nbr_new[k, gc]
                val = inb & (ns >= 0)
                coord = ns - (r * NS - conv_off - H)
                slots[k] = np.where(val, coord % R, R)
            out[r] = slots.reshape(K, nt, T // 16, 16).transpose(1, 3, 0, 2)
        return out.astype(np.int16)

    idx0 = idx_for(2 * H, NT0)
    idx1 = idx_for(H, NT1)
    idx2 = idx_for(0, NT2)

    w0b = _bf16(W0p)
    w1b = _bf16(W1)
    w2b = _bf16(W2p)
    wdb = _bf16(wd)

    in_maps = []
    for r in range(NC):
        lo = r * NS - 3 * H
        fw = np.zeros((LW, C), np.float32)
        a, b = max(0, lo), min(N, lo + LW)
        if b > a:
            fw[a - lo:b - lo] = feats_p[a:b]
        in_maps.append({
            "featsw": _bf16(fw),
            "idx0": np.ascontiguousarray(idx0[r]).reshape(NT0, 16, -1),
            "idx1": np.ascontiguousarray(idx1[r]).reshape(NT1, 16, -1),
            "idx2": np.ascontiguousarray(idx2[r]).reshape(NT2, 16, -1),
            "w0": w0b, "w1": w1b, "w2": w2b, "wd": wdb,
        })
    return in_maps, perm


def _build_graph():
    import concourse.bacc as bacc
    import concourse.mybir as mybir
    import concourse.tile as tile

    F32 = mybir.dt.float32
    BF16 = mybir.dt.bfloat16
    I16 = mybir.dt.int16
    NIDX = T // 16  # 56

    nc = bacc.Bacc(None, target_bir_lowering=False, debug=False,
                   num_swdge_queues=4)
    featsw = nc.declare_dram_parameter("featsw", [LW, C], BF16, isOutput=False)
    idx0 = nc.declare_dram_parameter("idx0", [NT0, 16, K * NIDX], I16, isOutput=False)
    idx1 = nc.declare_dram_parameter("idx1", [NT1, 16, K * NIDX], I16, isOutput=False)
    idx2 = nc.declare_dram_parameter("idx2", [NT2, 16, K * NIDX], I16, isOutput=False)
    w0 = nc.declare_dram_parameter("w0", [K, C, C], BF16, isOutput=False)
    w1 = nc.declare_dram_parameter("w1", [K, C, CO], BF16, isOutput=False)
    w2 = nc.declare_dram_parameter("w2", [K, C, CO], BF16, isOutput=False)
    wd = nc.declare_dram_parameter("wd", [C, CO], BF16, isOutput=False)
    out_ext = nc.declare_dram_parameter("out", [CO, L2], F32, isOutput=True)

    out0_dram = nc.dram_tensor("out0_dram", [L0, C], BF16)
    out1_dram = nc.dram_tensor("out1_dram", [L1, C], BF16)
    dT_dram = nc.dram_tensor("dT_dram", [CO, L2], F32)

    def ring_fill(ring, src_dram, row0, nrows, slot0):
        # tokens row0..row0+nrows from row-major DRAM into ring stripes
        s0 = slot0 // 128
        ns = nrows // 128
        nc.sync.dma_start(
            out=ring[:, s0 * 128:(s0 + ns) * 128].rearrange(
                "p (s e) -> p s e", e=128),
            in_=src_dram[row0:row0 + nrows].rearrange(
                "(s p) e -> p s e", p=128),
        )

    with tile.TileContext(nc) as tc:
        with (
            tc.tile_pool(name="pers", bufs=1) as pers,
            tc.tile_pool(name="ip", bufs=4) as ip,
            tc.tile_pool(name="gp", bufs=4) as gp,
            tc.tile_pool(name="st", bufs=3) as st,
            tc.tile_pool(name="oc", bufs=3) as ocp,
            tc.tile_pool(name="ps", bufs=1, space="PSUM") as psp,
            tc.tile_pool(name="psb", bufs=2, space="PSUM") as psbp,
        ):
            wsd = pers.tile([C, CO], BF16, tag="wsd")
            nc.sync.dma_start(out=wsd[:, :], in_=wd[:, :])

            def conv_phase(phase, nt, idx_t, ws, src_dram, cout, dst_dram):
                ring = pers.tile([128, (STR + 1) * 128], BF16, tag="ring")
                nc.vector.memset(ring[:, STR * 128:(STR + 1) * 128], 0.0)
                ring_fill(ring, src_dram, 0, SPAN, 0)
                for t in range(nt):
                    if t > 0:
                        c0 = SPAN + (t - 1) * T
                        ring_fill(ring, src_dram, c0, T, c0 % R)
                    idxs = ip.tile([128, K, NIDX], I16, tag="idx")
                    nc.sync.dma_start(
                        out=idxs[:, :, :].rearrange("p k s -> p (k s)"),
                        in_=idx_t[t].unsqueeze(0).broadcast_to([8, 16, K * NIDX]),
                    )
                    psum = psp.tile([cout, T], F32, tag=f"acc{min(phase,1)}")
                    for k in range(K):
                        g = gp.tile([128, 1, T], BF16, tag="g")
                        nc.gpsimd.dma_gather(
                            out_ap=g[:, :, :], in_ap=ring[:, :],
                            idxs_ap=idxs[:, k, :],
                            num_idxs=T, num_idxs_reg=T, elem_size=C,
                            transpose=True, queue_num=k % 4,
                            sbuf_tokens_per_rank=128,
                            sbuf_free_dim_per_rank=256,
                        )
                        for lo, hi in ((0, 512), (512, T)):
                            nc.tensor.matmul(
                                psum[:, lo:hi], ws[:, k, :], g[:, 0, lo:hi],
                                start=(k == 0), stop=(k == K - 1),
                            )
                        if phase == 0 and k == 13 and SKEW <= t < SKEW + NT2:
                            c = t - SKEW
                            psd = psbp.tile([CO, T], F32, tag="accd")
                            for lo, hi in ((0, 512), (512, T)):
                                nc.tensor.matmul(
                                    psd[:, lo:hi], wsd[:, :], g[:, 0, lo:hi],
                                    start=True, stop=True,
                                )
                            od = ocp.tile([CO, T], F32, tag="od")
                            nc.vector.tensor_copy(od[:, :], psd[:, :])
                            nc.sync.dma_start(
                                out=dT_dram[:, c * T:(c + 1) * T], in_=od[:, :])
                    if phase < 2:
                        # bf16 cast + XBAR transpose to row-major tokens
                        sb = st.tile([128, T], BF16, tag="stg")
                        if cout < 128:
                            nc.vector.memset(sb[:, :], 0.0)
                        nc.vector.tensor_copy(sb[0:cout, :], psum[:, :])
                        tok = st.tile([128, (T // 128) * 128], BF16, tag="tok")
                        for s in range(T // 128):
                            nc.sync.dma_start_transpose(
                                tok[:, s * 128:(s + 1) * 128],
                                sb[:, s * 128:(s + 1) * 128],
                            )
                        nc.sync.dma_start(
                            out=dst_dram[t * T:(t + 1) * T].rearrange(
                                "(s p) e -> p s e", p=128),
                            in_=tok[:, :].rearrange("p (s e) -> p s e", e=128),
                        )
                    else:
                        dtt = ocp.tile([CO, T], F32, tag="dtt")
                        nc.sync.dma_start(
                            out=dtt[:, :], in_=dT_dram[:, t * T:(t + 1) * T])
                        oc = ocp.tile([CO, T], F32, tag="oc")
                        nc.vector.tensor_add(oc[:, :], psum[:, :], dtt[:, :])
                        nc.sync.dma_start(
                            out=out_ext[:, t * T:(t + 1) * T], in_=oc[:, :])

            ws0 = pers.tile([C, K, C], BF16, tag="ws0")
            nc.sync.dma_start(out=ws0[:, :, :], in_=w0.ap().rearrange("k c m -> c k m"))
            conv_phase(0, NT0, idx0, ws0, featsw, C, out0_dram)

            ws1 = pers.tile([C, K, CO], BF16, tag="ws1")
            nc.sync.dma_start(out=ws1[:, :, :], in_=w1.ap().rearrange("k c m -> c k m"))
            conv_phase(1, NT1, idx1, ws1, out0_dram, CO, out1_dram)

            ws2 = pers.tile([C, K, CO], BF16, tag="ws2")
            nc.sync.dma_start(out=ws2[:, :, :], in_=w2.ap().rearrange("k c m -> c k m"))
            conv_phase(2, NT2, idx2, ws2, out1_dram, CO, None)

    nc.compile()
    return nc


def kernel(feats, W0, W1, W2, nbr_idx):
    global LAST_EXEC_NS, LAST_RESULTS
    from concourse.bass_utils import run_bass_kernel_spmd

    in_maps, perm = _preprocess(feats, W0, W1, W2, nbr_idx)
    nc = _build_graph()
    trace = os.environ.get("KERNEL_NO_TRACE", "") == ""
    res = run_bass_kernel_spmd(nc, in_maps, core_ids=list(range(NC)), trace=trace)
    LAST_EXEC_NS = res.exec_time_ns
    LAST_RESULTS = res

    out_p = np.empty((N, CO), np.float32)
    for r in range(NC):
        out_p[r * NS:(r + 1) * NS] = res.results[r]["out"][:, :NS].T
    out = np.empty((N, CO), np.float32)
    out[perm] = out_p
    return out
